# revision 6
# baseline (speedup 1.0000x reference)
"""Trainium2 Bass kernel for nn_BatchedTeacherPolicy.

2048 independent per-teacher MLPs (obs-norm -> 48->512->256->128->12,
ELU between layers, tanh at the end). Pure data parallel: 256 teachers
per NeuronCore across 8 cores, 2 groups of 128 teachers per core.

Strategy (v3):
- All weights fp16 on the host (harness gate is rel_err < 2e-2; fp16
  lands ~1e-3) -> halves HBM traffic, the roofline for this
  memory-bound problem.
- L0/L1/L2 all on TensorE as per-teacher self-loading [K x 128o] fp16
  matmuls (FWL engages at 128 weight columns), each teacher's output
  column accumulated in PSUM across i-chunks. Weights host-transposed
  to [group, ichunk, i, teacher, o] so DMAs are contiguous per
  partition and lhsT slices are step-1. The MLP stays in transposed
  [feature, teacher] space the whole way: layer l's PSUM output tile is
  exactly layer l+1's rhs after the fused bias+ELU pass, so the only
  PE transposes are x0 at entry and x3 before the DVE tail.
- Per-teacher biases + ELU applied on the transposed [o, t] tiles
  (bias tiles host-transposed too).
- L3 (128->12) on DVE (12 columns would waste PE weight loads), tanh
  on ACT, contiguous output DMA.
- DMA split across rings to avoid FIFO head-of-line stalls: sync ring
  carries W0T + W1T(ic 0/1), gpsimd ring W1T(ic 2/3) + W2T + W3,
  vector ring all bias tiles, scalar ring norm inputs + outputs.
"""

from contextlib import ExitStack

import numpy as np

import concourse.bass as bass
import concourse.bacc as bacc
import concourse.tile as tile
from concourse import mybir
from concourse.bass_utils import run_bass_kernel_spmd

N, OBS = 2048, 48
DIMS = [(512, 48), (256, 512), (128, 256), (12, 128)]  # (out, in) per layer
N_CORES = 8
NPC = N // N_CORES  # teachers per core
P = 128             # partitions = teachers per group
G = NPC // P        # groups per core
TB = 16             # teachers per PE weight DMA chunk
TC = P // TB        # t-chunks per group

O0 = DIMS[0][0]
O1, I1 = DIMS[1]
O2, I2 = DIMS[2]
O3, I3 = DIMS[3]
OC0 = O0 // P       # 4 output chunks for L0
IC1 = I1 // P       # 4 contraction chunks for L1
OC1 = O1 // P       # 2 output chunks for L1
IC2 = I2 // P       # 2 contraction chunks for L2

F32 = mybir.dt.float32
F16 = mybir.dt.float16
AF = mybir.ActivationFunctionType
ALU = mybir.AluOpType

_cached = {}


def _build_bass():
    nc = bacc.Bacc(trn_type="TRN2", target_bir_lowering=False)

    obs_d = nc.dram_tensor("obs", [NPC, OBS], F32, kind="ExternalInput")
    mean_d = nc.dram_tensor("mean", [NPC, OBS], F32, kind="ExternalInput")
    std_d = nc.dram_tensor("std", [NPC, OBS], F32, kind="ExternalInput")
    w0t_d = nc.dram_tensor("W0T", [G, OBS, P, O0], F16, kind="ExternalInput")
    b0t_d = nc.dram_tensor("b0T", [G, O0, P], F32, kind="ExternalInput")
    w1t_d = nc.dram_tensor("W1T", [G, IC1, P, P, O1], F16, kind="ExternalInput")
    b1t_d = nc.dram_tensor("b1T", [G, O1, P], F32, kind="ExternalInput")
    w2t_d = nc.dram_tensor("W2T", [G, IC2, P, P, O2], F16, kind="ExternalInput")
    b2t_d = nc.dram_tensor("b2T", [G, O2, P], F32, kind="ExternalInput")
    w3_d = nc.dram_tensor("W3", [NPC, O3, I3], F16, kind="ExternalInput")
    b3_d = nc.dram_tensor("b3", [NPC, O3], F32, kind="ExternalInput")
    out_d = nc.dram_tensor("out", [NPC, O3], F32, kind="ExternalOutput")

    from concourse.masks import make_identity

    with ExitStack() as ctx:
        tc = ctx.enter_context(tile.TileContext(nc))
        wpool = ctx.enter_context(tc.tile_pool(name="wpool", bufs=2))
        xpool = ctx.enter_context(tc.tile_pool(name="xpool", bufs=2))
        spool = ctx.enter_context(tc.tile_pool(name="spool", bufs=2))
        bpool = ctx.enter_context(tc.tile_pool(name="bpool", bufs=2))
        ppool = ctx.enter_context(tc.tile_pool(name="ppool", bufs=2, space="PSUM"))
        ipool = ctx.enter_context(tc.tile_pool(name="ipool", bufs=1))

        ident = ipool.tile([P, P], F16)
        make_identity(nc, ident)

        def emit_norm(g):
            n0 = g * P
            # x0 = clip((obs - mean)/std, -5, 5), cast to fp16
            obs_t = spool.tile([P, OBS], F32, tag="nrm")
            nc.scalar.dma_start(out=obs_t, in_=obs_d[n0 : n0 + P, :])
            mean_t = spool.tile([P, OBS], F32, tag="nrm")
            nc.scalar.dma_start(out=mean_t, in_=mean_d[n0 : n0 + P, :])
            std_t = spool.tile([P, OBS], F32, tag="nrm")
            nc.scalar.dma_start(out=std_t, in_=std_d[n0 : n0 + P, :])

            # Each DVE op may carry at most ONE new semaphore wait (TRN2
            # TT-struct limit), so feed multi-operand ops through
            # single-input ops that absorb the DMA waits first.
            nmean = spool.tile([P, OBS], F32, tag="nmean")
            nc.vector.tensor_scalar_mul(nmean, mean_t, -1.0)
            rstd = spool.tile([P, OBS], F32, tag="rstd")
            nc.vector.reciprocal(rstd, std_t)
            x = spool.tile([P, OBS], F32, tag="x0f", name=f"x0f_{g}")
            nc.vector.tensor_add(x, obs_t, nmean)
            nc.vector.tensor_mul(x, x, rstd)
            x_h = xpool.tile([P, OBS], F16, tag="x0h", name=f"x0h_{g}")
            nc.vector.tensor_scalar(
                out=x_h, in0=x, scalar1=-5.0, scalar2=5.0,
                op0=ALU.max, op1=ALU.min,
            )
            return x_h

        def emit_elu_tile(g, tag, ps_ap, bt, out_h):
            """out_h (fp16) = ELU(ps_ap + bt) for one [128, 128] transposed
            tile; ps_ap is PSUM fp32, bt a [128,128] f32 SBUF bias tile."""
            yb = spool.tile([P, P], F32, tag="yb", name=f"yb_{tag}_{g}")
            nc.vector.tensor_add(yb, ps_ap, bt)
            m_h = spool.tile([P, P], F16, tag="eluh", name=f"m_{tag}_{g}")
            nc.vector.tensor_scalar_min(m_h, yb, 0.0)
            e_h = spool.tile([P, P], F16, tag="eluh", name=f"e_{tag}_{g}")
            nc.scalar.activation(e_h, m_h, AF.Exp)
            # (max(yb,0) + e) - 1
            nc.vector.scalar_tensor_tensor(
                out=out_h, in0=yb, scalar=0.0, in1=e_h,
                op0=ALU.max, op1=ALU.add,
            )
            nc.vector.tensor_scalar_add(out_h, out_h, -1.0)

        def emit_group(g):
            n0 = g * P

            # All bias tiles up front on the (otherwise idle) vector ring.
            b0ts = []
            for oc in range(OC0):
                bt = bpool.tile([P, P], F32, tag="b0t", bufs=2 * OC0,
                                name=f"b0t_{g}_{oc}")
                nc.scalar.dma_start(out=bt, in_=b0t_d[g, oc * P : (oc + 1) * P, :])
                b0ts.append(bt)
            b1ts = []
            for oc in range(OC1):
                bt = bpool.tile([P, P], F32, tag="b1t", bufs=2 * OC1,
                                name=f"b1t_{g}_{oc}")
                nc.scalar.dma_start(out=bt, in_=b1t_d[g, oc * P : (oc + 1) * P, :])
                b1ts.append(bt)
            bt2 = bpool.tile([P, P], F32, tag="b2t", name=f"b2t_{g}")
            nc.scalar.dma_start(out=bt2, in_=b2t_d[g, :, :])
            bt3 = bpool.tile([P, O3], F32, tag="b3", name=f"b3_{g}")
            nc.scalar.dma_start(out=bt3, in_=b3_d[n0 : n0 + P, :])

            x_h = emit_norm(g)

            # ---- transpose x0 -> x0t [48 i, 128 t] fp16 ----
            pst0 = ppool.tile([OBS, P], F16, tag="pst", name=f"pst0_{g}")
            nc.tensor.transpose(pst0, x_h, ident)
            x0t = xpool.tile([OBS, P], F16, tag="x0t", name=f"x0t_{g}")
            nc.scalar.copy(x0t, pst0)

            # ---- L0 on PE: y0ps[o, t], single K=48 chunk per MM ----
            y0ps = ppool.tile([P, OC0, P], F32, tag="y0ps", name=f"y0ps_{g}")
            for tcn in range(TC):
                t0 = tcn * TB
                w0 = wpool.tile([OBS, TB, O0], F16, tag="w0", bufs=4,
                                name=f"w0_{g}_{tcn}")
                nc.sync.dma_start(out=w0, in_=w0t_d[g, :, t0 : t0 + TB, :])
                for tl in range(TB):
                    t = t0 + tl
                    for oc in range(OC0):
                        nc.tensor.matmul(
                            y0ps[:, oc, t : t + 1],
                            lhsT=w0[:, tl, oc * P : (oc + 1) * P],
                            rhs=x0t[:, t : t + 1],
                            start=True,
                            stop=True,
                        )

            # ---- L0 bias + ELU on transposed tiles -> x1t[ic] ----
            x1t = []
            for ic in range(OC0):
                xt = xpool.tile([P, P], F16, tag="x1t", bufs=2 * OC0,
                                name=f"x1t_{g}_{ic}")
                emit_elu_tile(g, f"l0_{ic}", y0ps[:, ic, :], b0ts[ic], xt)
                x1t.append(xt)

            # ---- L1 on PE: yps[o, t] accumulated over 4 i-chunks ----
            yps = ppool.tile([P, OC1, P], F32, tag="yps", name=f"yps_{g}")
            for tcn in range(TC):
                t0 = tcn * TB
                w1s = []
                for ic in range(IC1):
                    w1 = wpool.tile([P, TB, O1], F16, tag="w1", bufs=10,
                                    name=f"w1_{g}_{tcn}_{ic}")
                    eng = nc.sync if ic < 2 else nc.gpsimd
                    eng.dma_start(out=w1, in_=w1t_d[g, ic, :, t0 : t0 + TB, :])
                    w1s.append(w1)
                for tl in range(TB):
                    t = t0 + tl
                    for oc in range(OC1):
                        for ic in range(IC1):
                            nc.tensor.matmul(
                                yps[:, oc, t : t + 1],
                                lhsT=w1s[ic][:, tl, oc * P : (oc + 1) * P],
                                rhs=x1t[ic][:, t : t + 1],
                                start=(ic == 0),
                                stop=(ic == IC1 - 1),
                            )

            # ---- L1 bias + ELU -> x2t[ic] ----
            x2t = []
            for oc in range(OC1):
                xt = xpool.tile([P, P], F16, tag="x2t", bufs=2 * OC1,
                                name=f"x2t_{g}_{oc}")
                emit_elu_tile(g, f"l1_{oc}", yps[:, oc, :], b1ts[oc], xt)
                x2t.append(xt)

            # ---- L2 on PE ----
            y2ps = ppool.tile([P, P], F32, tag="y2ps", name=f"y2ps_{g}")
            for tcn in range(TC):
                t0 = tcn * TB
                w2s = []
                for ic in range(IC2):
                    w2 = wpool.tile([P, TB, O2], F16, tag="w2", bufs=8,
                                    name=f"w2_{g}_{tcn}_{ic}")
                    nc.gpsimd.dma_start(out=w2, in_=w2t_d[g, ic, :, t0 : t0 + TB, :])
                    w2s.append(w2)
                for tl in range(TB):
                    t = t0 + tl
                    for ic in range(IC2):
                        nc.tensor.matmul(
                            y2ps[:, t : t + 1],
                            lhsT=w2s[ic][:, tl, :],
                            rhs=x2t[ic][:, t : t + 1],
                            start=(ic == 0),
                            stop=(ic == IC2 - 1),
                        )

            # ---- L2 bias + ELU -> x3t fp16 [128 o2, 128 t] ----
            x3t = xpool.tile([P, P], F16, tag="x3t", name=f"x3t_{g}")
            emit_elu_tile(g, "l2", y2ps, bt2, x3t)

            # ---- transpose back -> x3 [t, i] ----
            pst2 = ppool.tile([P, P], F16, tag="pst", name=f"pst2_{g}")
            nc.tensor.transpose(pst2, x3t, ident)
            x3_h = xpool.tile([P, P], F16, tag="x3h", name=f"x3h_{g}")
            nc.scalar.copy(x3_h, pst2)

            # ---- L3 on DVE (12 outputs) + tanh ----
            w3t = wpool.tile([P, O3, I3], F16, tag="w3", name=f"w3_{g}")
            nc.gpsimd.dma_start(out=w3t, in_=w3_d[n0 : n0 + P, :, :])
            y3 = spool.tile([P, O3], F32, tag="y3", name=f"y3_{g}")
            scr = spool.tile([P, I3], F16, tag="scr", name=f"scr_{g}")
            for o in range(O3):
                nc.vector.affine_mul_reduce(
                    out=scr,
                    accum_out=y3[:, o : o + 1],
                    in0=w3t[:, o, :],
                    in1=x3_h,
                    scale=1.0,
                    bias=0.0,
                )
            nc.vector.tensor_add(y3, y3, bt3)
            yt = spool.tile([P, O3], F32, tag="yt", name=f"yt_{g}")
            nc.scalar.activation(yt, y3, AF.Tanh)
            nc.scalar.dma_start(out=out_d[n0 : n0 + P, :], in_=yt)

        for g in range(G):
            emit_group(g)

    nc.compile()
    return nc


def _get_nc():
    if "nc" not in _cached:
        _cached["nc"] = _build_bass()
    return _cached["nc"]


def _pack_core_inputs(full, c):
    """Shard + lay out one core's inputs (fp16 weights, PE-transposed
    W0/W1/W2 and biases)."""
    sl = slice(c * NPC, (c + 1) * NPC)
    f16 = np.float16
    m = {
        "obs": np.ascontiguousarray(full["obs"][sl]),
        "mean": np.ascontiguousarray(full["mean"][sl]),
        "std": np.ascontiguousarray(full["std"][sl]),
        "W3": np.ascontiguousarray(full["W3"][sl].astype(f16)),
        "b3": np.ascontiguousarray(full["b3"][sl]),
    }
    # W0T[g, i, t, o] = W0[g*128+t, o, i]
    w0c = full["W0"][sl].astype(f16)  # [NPC, 512, 48]
    m["W0T"] = np.ascontiguousarray(
        w0c.reshape(G, P, O0, OBS).transpose(0, 3, 1, 2)
    )
    m["b0T"] = np.ascontiguousarray(
        full["b0"][sl].reshape(G, P, O0).transpose(0, 2, 1)
    )
    # W1T[g, ic, i, t, o] = W1[g*128+t, o, ic*128+i]
    w1c = full["W1"][sl].astype(f16)  # [NPC, 256, 512]
    m["W1T"] = np.ascontiguousarray(
        w1c.reshape(G, P, O1, IC1, P).transpose(0, 3, 4, 1, 2)
    )
    m["b1T"] = np.ascontiguousarray(
        full["b1"][sl].reshape(G, P, O1).transpose(0, 2, 1)
    )
    # W2T[g, ic, i, t, o] = W2[g*128+t, o, ic*128+i]
    w2c = full["W2"][sl].astype(f16)  # [NPC, 128, 256]
    m["W2T"] = np.ascontiguousarray(
        w2c.reshape(G, P, O2, IC2, P).transpose(0, 3, 4, 1, 2)
    )
    m["b2T"] = np.ascontiguousarray(
        full["b2"][sl].reshape(G, P, O2).transpose(0, 2, 1)
    )
    return m


def kernel(obs, mean, std, W0, b0, W1, b1, W2, b2, W3, b3, _trace=False):
    nc = _get_nc()
    full = {
        "obs": np.asarray(obs), "mean": np.asarray(mean), "std": np.asarray(std),
        "W0": np.asarray(W0), "b0": np.asarray(b0),
        "W1": np.asarray(W1), "b1": np.asarray(b1),
        "W2": np.asarray(W2), "b2": np.asarray(b2),
        "W3": np.asarray(W3), "b3": np.asarray(b3),
    }
    in_maps = [_pack_core_inputs(full, c) for c in range(N_CORES)]
    res = run_bass_kernel_spmd(
        nc, in_maps, core_ids=list(range(N_CORES)), trace=_trace
    )
    _cached["last_results"] = res
    out = np.concatenate([res.results[c]["out"] for c in range(N_CORES)], axis=0)
    return out
